# revision 5
# baseline (speedup 1.0000x reference)
"""Trainium2 Bass kernel for nn_ClebschGordanDecomposer (sparse Lie bracket).

reference:
    contrib = v1[:, I] * v2[:, J] * C          # [B, NNZ]
    antisym = zeros.at[:, K].add(contrib)      # [B, D]
    sym = v1 * v2                              # [B, D]
    scalar = sum(sym, -1, keepdims=True)       # [B, 1]

Strategy (data-parallel over B across 8 cores, "T layout" on device):
  - Host transposes each B-shard to [D, BS] so the batch lies on the SBUF
    free dim and the algebra dim d on partitions; outputs are produced
    transposed and the host transposes them back. All arithmetic is on-device.
  - The COO triples (I,J,K,C) are known at kernel-build time, so the
    gather/scatter structure is compiled into one-hot selection matrices and
    executed as PE matmuls (float32r, ~1 cycle/row, TF32-ish precision):
        S[t, b]   = v1[I_t, b] + v2[J_t, b]      (gather-sum, PE accumulate)
        Ssq       = S^2                           (ACT Square on PSUM evac)
        antisymT  = scatterK(C/2 * Ssq) - W1q @ (v1^2) - W2q @ (v2^2)
    using v1_i*v2_j = ((v1_i+v2_j)^2 - v1_i^2 - v2_j^2)/2; the quadratic
    correction terms collapse into precomputed dense [D, D] matrices
    (W1q[d,k] = sum_t C_t/2 [I_t=d][K_t=k], W2q likewise with J).
  - Triples are lex-sorted by (K half, I half, J half) so each 128-wide
    t-chunk touches a single 128-partition block of v1T, v2T and of the
    antisym accumulator: one matmul per operand per chunk.
  - sym runs on DVE; scalar = ones-vector matmul over symT.
"""

import numpy as np

import concourse.bass as bass
import concourse.tile as tile
from concourse import bacc, mybir
from concourse.bass_utils import run_bass_kernel_spmd

B, D, NNZ, NCORES = 65536, 248, 4096, 8
BS = B // NCORES  # 8192 rows per core
NT = 512  # batch columns per tile
D0 = 128  # first partition block of d
D1 = D - D0  # 120
CH = 128  # COO triples per chunk
F32R = mybir.dt.float32r
F32 = mybir.dt.float32


# ---------------------------------------------------------------- host prep


def _plan_chunks(I, J, K):
    """Lex-sort triples by (K//128, I//128, J//128); pad each of the 8 cells
    to a multiple of CH. Returns (perm, valid, meta) where perm[i] indexes the
    original triple for padded slot i (padding slots repeat index 0 with
    valid=False), and meta is a list of (ihalf, jhalf, khalf) per chunk."""
    ih, jh, kh = I // D0, J // D0, K // D0
    cell = ((kh * 2 + ih) * 2 + jh).astype(np.int64)
    order = np.lexsort((np.arange(NNZ), cell))
    perm_list, valid_list, meta = [], [], []
    for cell_id in range(8):
        sel = order[cell[order] == cell_id]
        if len(sel) == 0:
            continue
        pad = (-len(sel)) % CH
        padded = np.concatenate([sel, np.zeros(pad, dtype=np.int64)])
        pvalid = np.concatenate(
            [np.ones(len(sel), dtype=bool), np.zeros(pad, dtype=bool)]
        )
        khalf, ihalf, jhalf = cell_id // 4, (cell_id // 2) % 2, cell_id % 2
        for c in range(len(padded) // CH):
            perm_list.append(padded[c * CH : (c + 1) * CH])
            valid_list.append(pvalid[c * CH : (c + 1) * CH])
            meta.append((ihalf, jhalf, khalf))
    return np.concatenate(perm_list), np.concatenate(valid_list), meta


def _build_weights(I, J, K, C):
    I = np.asarray(I).astype(np.int64)
    J = np.asarray(J).astype(np.int64)
    K = np.asarray(K).astype(np.int64)
    C = np.asarray(C).astype(np.float32)

    perm, valid, meta = _plan_chunks(I, J, K)
    nch = len(meta)
    Ip, Jp, Kp, Cp = I[perm], J[perm], K[perm], C[perm] * valid

    # Gather selection weights: wi[d_local, c*CH + m] = 1 where
    # Ip[c*CH+m] == ihalf*128 + d_local (valid slots only). One [<=128, CH]
    # block per chunk, all stored in a single [128, nch*CH] tensor.
    wi = np.zeros((D0, nch * CH), dtype=np.float32)
    wj = np.zeros((D0, nch * CH), dtype=np.float32)
    wkc = np.zeros((CH, nch * CH), dtype=np.float32)
    for c, (ihalf, jhalf, khalf) in enumerate(meta):
        sl = slice(c * CH, (c + 1) * CH)
        m = np.arange(CH)
        v = valid[sl]
        ri = np.where(v, Ip[sl] - ihalf * D0, 0)
        rj = np.where(v, Jp[sl] - jhalf * D0, 0)
        rk = np.where(v, Kp[sl] - khalf * D0, 0)
        wi[ri, c * CH + m] = v.astype(np.float32)
        wj[rj, c * CH + m] = v.astype(np.float32)
        wkc[m, c * CH + rk] = 0.5 * Cp[sl]

    # Dense quadratic corrections: w1q[d, k] = -1/2 sum_t C_t [I_t=d][K_t=k]
    w1q = np.zeros((D, D), dtype=np.float32)
    w2q = np.zeros((D, D), dtype=np.float32)
    np.add.at(w1q, (I, K), -0.5 * C)
    np.add.at(w2q, (J, K), -0.5 * C)

    ones0 = np.ones((D0, 1), dtype=np.float32)
    ones1 = np.ones((D1, 1), dtype=np.float32)
    return {
        "wi": wi,
        "wj": wj,
        "wkc": wkc,
        "w1q0": np.ascontiguousarray(w1q[:D0]),
        "w1q1": np.ascontiguousarray(w1q[D0:]),
        "w2q0": np.ascontiguousarray(w2q[:D0]),
        "w2q1": np.ascontiguousarray(w2q[D0:]),
        "ones0": ones0,
        "ones1": ones1,
    }, meta


# ------------------------------------------------------------- bass program


def _emit(tc, aps, meta, bs, ctx):
    nc = tc.nc
    nch = len(meta)
    ntiles = bs // NT
    BLK = 2  # chunks squared per ACT op: S psum tile is [128, BLK*NT]

    wpool = ctx.enter_context(tc.tile_pool(name="w", bufs=1))
    io = ctx.enter_context(tc.tile_pool(name="io", bufs=2))
    mid = ctx.enter_context(tc.tile_pool(name="mid", bufs=2))
    ps_s = ctx.enter_context(tc.tile_pool(name="ps_s", bufs=2, space="PSUM"))
    ps_a = ctx.enter_context(tc.tile_pool(name="ps_a", bufs=2, space="PSUM"))

    # --- resident weights
    w = {}
    for name, parts, width in [
        ("wi", D0, nch * CH),
        ("wj", D0, nch * CH),
        ("wkc", CH, nch * CH),
        ("w1q0", D0, D),
        ("w1q1", D1, D),
        ("w2q0", D0, D),
        ("w2q1", D1, D),
        ("ones0", D0, 1),
        ("ones1", D1, 1),
    ]:
        t = wpool.tile([parts, width], F32R, tag=name)
        nc.sync.dma_start(t[:], aps[name][:])
        w[name] = t

    for n in range(ntiles):
        nsl = slice(n * NT, (n + 1) * NT)

        # --- load v1T/v2T halves
        v1t0 = io.tile([D0, NT], F32R, tag="v1t0")
        v1t1 = io.tile([D1, NT], F32R, tag="v1t1")
        v2t0 = io.tile([D0, NT], F32R, tag="v2t0")
        v2t1 = io.tile([D1, NT], F32R, tag="v2t1")
        nc.sync.dma_start(v1t0[:], aps["v1t"][:D0, nsl])
        nc.sync.dma_start(v1t1[:], aps["v1t"][D0:, nsl])
        nc.sync.dma_start(v2t0[:], aps["v2t"][:D0, nsl])
        nc.sync.dma_start(v2t1[:], aps["v2t"][D0:, nsl])
        vt = {(1, 0): v1t0, (1, 1): v1t1, (2, 0): v2t0, (2, 1): v2t1}
        v1f0, v1f1 = v1t0[:].bitcast(F32), v1t1[:].bitcast(F32)
        v2f0, v2f1 = v2t0[:].bitcast(F32), v2t1[:].bitcast(F32)

        # --- sym (exact fp32) + f32r copy for the scalar matmul
        sym0 = mid.tile([D0, NT], F32, tag="sym0")
        sym1 = mid.tile([D1, NT], F32, tag="sym1")
        nc.vector.tensor_tensor(sym0[:], v1f0, v2f0, op=mybir.AluOpType.mult)
        nc.vector.tensor_tensor(sym1[:], v1f1, v2f1, op=mybir.AluOpType.mult)
        nc.sync.dma_start(aps["symT"][:D0, nsl], sym0[:])
        nc.sync.dma_start(aps["symT"][D0:, nsl], sym1[:])
        sym0r = mid.tile([D0, NT], F32R, tag="sym0r")
        sym1r = mid.tile([D1, NT], F32R, tag="sym1r")
        nc.vector.tensor_copy(sym0r[:], sym0[:])
        nc.vector.tensor_copy(sym1r[:], sym1[:])

        # --- q tiles (squares of inputs, f32r)
        q10 = mid.tile([D0, NT], F32R, tag="q10")
        q11 = mid.tile([D1, NT], F32R, tag="q11")
        q20 = mid.tile([D0, NT], F32R, tag="q20")
        q21 = mid.tile([D1, NT], F32R, tag="q21")
        nc.vector.tensor_tensor(q10[:], v1f0, v1f0, op=mybir.AluOpType.mult)
        nc.vector.tensor_tensor(q11[:], v1f1, v1f1, op=mybir.AluOpType.mult)
        nc.vector.tensor_tensor(q20[:], v2f0, v2f0, op=mybir.AluOpType.mult)
        nc.vector.tensor_tensor(q21[:], v2f1, v2f1, op=mybir.AluOpType.mult)

        # --- antisym accumulators; quadratic corrections start the groups
        anti0 = ps_a.tile([D0, NT], F32, tag="anti0", space="PSUM")
        anti1 = ps_a.tile([D1, NT], F32, tag="anti1", space="PSUM")
        nc.tensor.matmul(anti0[:], w["w1q0"][:, :D0], q10[:], start=True, stop=False)
        nc.tensor.matmul(anti0[:], w["w1q1"][:, :D0], q11[:], start=False, stop=False)
        nc.tensor.matmul(anti0[:], w["w2q0"][:, :D0], q20[:], start=False, stop=False)
        nc.tensor.matmul(anti0[:], w["w2q1"][:, :D0], q21[:], start=False, stop=False)
        nc.tensor.matmul(anti1[:], w["w1q0"][:, D0:], q10[:], start=True, stop=False)
        nc.tensor.matmul(anti1[:], w["w1q1"][:, D0:], q11[:], start=False, stop=False)
        nc.tensor.matmul(anti1[:], w["w2q0"][:, D0:], q20[:], start=False, stop=False)
        nc.tensor.matmul(anti1[:], w["w2q1"][:, D0:], q21[:], start=False, stop=False)
        anti = {0: anti0, 1: anti1}
        last_k = {0: None, 1: None}
        for c, (_, _, khalf) in enumerate(meta):
            last_k[khalf] = c

        # --- gather-sum chunks, square, scatter
        for c0 in range(0, nch, BLK):
            cn = min(BLK, nch - c0)
            spsum = ps_s.tile([CH, BLK * NT], F32, tag="s", space="PSUM")
            for ci in range(cn):
                c = c0 + ci
                ihalf, jhalf, khalf = meta[c]
                col = spsum[:, ci * NT : (ci + 1) * NT]
                csl = slice(c * CH, (c + 1) * CH)
                dlim = D0 if ihalf == 0 else D1
                nc.tensor.matmul(
                    col,
                    w["wi"][:dlim, csl],
                    vt[(1, ihalf)][:],
                    start=True,
                    stop=False,
                )
                dlim = D0 if jhalf == 0 else D1
                nc.tensor.matmul(
                    col,
                    w["wj"][:dlim, csl],
                    vt[(2, jhalf)][:],
                    start=False,
                    stop=True,
                )
            ssq = mid.tile([CH, BLK * NT], F32R, tag="ssq")
            nc.scalar.activation(
                ssq[:, : cn * NT],
                spsum[:, : cn * NT],
                mybir.ActivationFunctionType.Square,
            )
            for ci in range(cn):
                c = c0 + ci
                ihalf, jhalf, khalf = meta[c]
                klim = D0 if khalf == 0 else D1
                nc.tensor.matmul(
                    anti[khalf][:],
                    w["wkc"][:, c * CH : c * CH + klim],
                    ssq[:, ci * NT : (ci + 1) * NT],
                    start=False,
                    stop=(c == last_k[khalf]),
                )

        # --- scalar = ones^T @ symT
        scal = ps_a.tile([1, NT], F32, tag="anti0", space="PSUM")
        nc.tensor.matmul(scal[:], w["ones0"][:], sym0r[:], start=True, stop=False)
        nc.tensor.matmul(scal[:], w["ones1"][:], sym1r[:], start=False, stop=True)
        scal_sb = mid.tile([1, NT], F32, tag="scal")
        nc.vector.tensor_copy(scal_sb[:], scal[:])
        nc.sync.dma_start(aps["scalT"][:, nsl], scal_sb[:])

        # --- evacuate antisym
        a0 = mid.tile([D0, NT], F32, tag="a0")
        a1 = mid.tile([D1, NT], F32, tag="a1")
        nc.vector.tensor_copy(a0[:], anti0[:])
        nc.vector.tensor_copy(a1[:], anti1[:])
        nc.sync.dma_start(aps["antiT"][:D0, nsl], a0[:])
        nc.sync.dma_start(aps["antiT"][D0:, nsl], a1[:])


def build_program(meta, nch, bs=BS, num_devices=NCORES):
    nc = bacc.Bacc(
        "TRN2", target_bir_lowering=False, debug=False, num_devices=num_devices
    )
    aps = {}
    for name, shape, dt, kind in [
        ("v1t", [D, bs], F32R, "ExternalInput"),
        ("v2t", [D, bs], F32R, "ExternalInput"),
        ("wi", [D0, nch * CH], F32R, "ExternalInput"),
        ("wj", [D0, nch * CH], F32R, "ExternalInput"),
        ("wkc", [CH, nch * CH], F32R, "ExternalInput"),
        ("w1q0", [D0, D], F32R, "ExternalInput"),
        ("w1q1", [D1, D], F32R, "ExternalInput"),
        ("w2q0", [D0, D], F32R, "ExternalInput"),
        ("w2q1", [D1, D], F32R, "ExternalInput"),
        ("ones0", [D0, 1], F32R, "ExternalInput"),
        ("ones1", [D1, 1], F32R, "ExternalInput"),
        ("antiT", [D, bs], F32, "ExternalOutput"),
        ("symT", [D, bs], F32, "ExternalOutput"),
        ("scalT", [1, bs], F32, "ExternalOutput"),
    ]:
        aps[name] = nc.dram_tensor(name, shape, dt, kind=kind).ap()

    from contextlib import ExitStack

    with tile.TileContext(nc) as tc, ExitStack() as ctx:
        _emit(tc, aps, meta, bs, ctx)
    nc.compile()
    return nc


_CACHE = {}


def kernel(v1, v2, I, J, K, C, _bs=BS, _trace=False):
    v1 = np.asarray(v1, dtype=np.float32)
    v2 = np.asarray(v2, dtype=np.float32)
    weights, meta = _build_weights(I, J, K, C)
    nch = len(meta)

    ncores = (v1.shape[0] + _bs - 1) // _bs
    key = (nch, _bs, ncores)
    if key not in _CACHE:
        _CACHE[key] = build_program(meta, nch, bs=_bs, num_devices=ncores)
    nc = _CACHE[key]

    in_maps = []
    for s in range(ncores):
        sl = slice(s * _bs, (s + 1) * _bs)
        m = {
            "v1t": np.ascontiguousarray(v1[sl].T),
            "v2t": np.ascontiguousarray(v2[sl].T),
        }
        m.update(weights)
        in_maps.append(m)

    res = run_bass_kernel_spmd(
        nc, in_maps, core_ids=list(range(ncores)), trace=_trace
    )
    anti = np.concatenate([r["antiT"].T for r in res.results], axis=0)
    sym = np.concatenate([r["symT"].T for r in res.results], axis=0)
    scal = np.concatenate([r["scalT"].T for r in res.results], axis=0)
    out = (anti, sym, scal)
    if _trace:
        return out, res
    return out
